# revision 15
# baseline (speedup 1.0000x reference)
"""Trainium2 8-core kernel for nn_Attention_55070070670307.

Reference model: per-head Cayley-orthogonalized projections (OrthogonLin)
feeding standard multi-head softmax attention.

  x: (2, 4096, 512) f32, 8 heads x 64 dim, Wq/Wk/Wv/Wo (512,512) + scalars
  aq/ak/av/ao + bias bo.

Strategy:
  * Host: Cayley-orthogonalize the four weight matrices per head (32 tiny
    64x64 solves -- negligible FLOPs, done in float64 numpy).
  * Device sharding: batch-parallel x head-parallel. Core c handles batch
    b = c//4 and heads {2*(c%4), 2*(c%4)+1}. Each core computes q/k/v
    projections for its 2 heads over the whole sequence (4096), full
    softmax attention per head, and the partial output projection
    (contribution of its 128 head-dims to all 512 output features).
  * The 4 cores of each batch group ReduceScatter the partial outputs
    (per 512-row chunk, overlapped with remaining compute), add bias,
    and write disjoint row-slices of the final output.

Device layouts (per core):
  xT   (512, 4096)  x[b] transposed (feature-major)       -> bf16 on chip
  qT/kT (128, 4096)  per-head-dim-major projections, bf16
  v    32 tiles (128n, 130) = [v_h0 | ones | v_h1 | ones] bf16 (ones col
       gives the softmax row-sum for free during the AV matmul)
  scores are computed transposed: sT (128k, 512q) = K_tile @ qT so that
  exp(sT) tiles feed the AV matmul as lhsT with zero transposes.
  Softmax uses the unnormalized trick: out = (exp(s) @ [v|1]); divide by
  the ones-column afterwards. No max-subtraction (scores*0.125 is in
  [-6, 6] comfortably for exp in f32).
"""

import os
import sys

import numpy as np

sys.path.insert(0, "/opt/trn_rl_repo")

HEADS = 8
DIM = 512
DH = 64  # dim per head
N = 4096  # sequence length
B = 2
SCALE = DH ** -0.5
NCORES = 8

F32 = None  # set lazily after mybir import
BF16 = None

_CACHE = {}
LAST_RESULT = None  # BassKernelResults of the most recent run (for test.py)


# ----------------------------------------------------------------------------
# Host-side Cayley orthogonalization (matches reference.cayley_heads, f64)
# ----------------------------------------------------------------------------
def cayley_heads_np(W: np.ndarray, alpha: float) -> np.ndarray:
    W = W.astype(np.float64)
    out, inn = W.shape
    d = inn // HEADS
    Wh = W.reshape(HEADS, d, inn)
    norms = np.sqrt((Wh * Wh).sum(axis=(1, 2), keepdims=True))
    Wn = float(alpha) * Wh / norms
    blocks = []
    I = np.eye(d)
    for j in range(HEADS):
        Wt = Wn[j].T  # (inn, d)
        U, V = Wt[:d], Wt[d:]
        A = U - U.T + V.T @ V
        IpA = I + A
        top = np.linalg.solve(IpA, I - A)
        bot = -2.0 * np.linalg.solve(IpA.T, V.T).T
        blocks.append(np.concatenate([top, bot], axis=0).T)  # (d, inn)
    return np.concatenate(blocks, axis=0)  # (out, inn) f64


# ----------------------------------------------------------------------------
# Device kernel builder (one SPMD graph, 8 cores)
# ----------------------------------------------------------------------------
def _build(rs_mode="chunked", reps=1, front_split=False, warm_table=True,
           pipelined_tail=False, inject=False, bcast="pe", es_bufs=3, fo_bufs=3, act2048=False, hybrid_exp=False, deep_bufs=False, w512=False):
    from concourse import bass, bacc, tile
    import concourse.mybir as mybir

    F32 = mybir.dt.float32
    BF16 = mybir.dt.bfloat16
    EXP = mybir.ActivationFunctionType.Exp

    nc = bacc.Bacc(None, target_bir_lowering=False, debug=False, num_devices=NCORES)

    xT_e = nc.declare_dram_parameter("xT", [DIM, N], F32, isOutput=False)
    wq_e = nc.declare_dram_parameter("wq", [DIM, 128], F32, isOutput=False)
    wk_e = nc.declare_dram_parameter("wk", [DIM, 128], F32, isOutput=False)
    wv_e = nc.declare_dram_parameter("wv", [DIM, 128], F32, isOutput=False)
    wo_e = nc.declare_dram_parameter("wo", [128, DIM], F32, isOutput=False)
    bo_e = nc.declare_dram_parameter("bo", [1, DIM], F32, isOutput=False)
    out_e = nc.declare_dram_parameter("out", [8, 128, DIM], F32, isOutput=True)

    NKT = N // 128        # 32 k tiles
    NQB = N // 512        # 8 q blocks (512 wide)
    VW = 130              # v tile width: 64 + 1 + 64 + 1
    PS_O_BUFS = 3 if pipelined_tail else 2
    PS_F_BUFS = 1 if pipelined_tail else 2
    SHARE_PF = act2048 or deep_bufs
    PS_BIG_BUFS = 3 if deep_bufs else 2

    import contextlib
    with tile.TileContext(nc) as tc:
        with contextlib.ExitStack() as stk:
          persist = stk.enter_context(tc.tile_pool(name="persist", bufs=1))
          stage = stk.enter_context(tc.tile_pool(name="stage", bufs=2))
          esp = stk.enter_context(tc.tile_pool(name="es", bufs=es_bufs))
          small = stk.enter_context(tc.tile_pool(name="small", bufs=3))
          fop = stk.enter_context(tc.tile_pool(name="fo", bufs=fo_bufs))
          ps_big = stk.enter_context(tc.tile_pool(name="ps_big", bufs=PS_BIG_BUFS, space="PSUM"))
          ps_o = stk.enter_context(tc.tile_pool(name="ps_o", bufs=PS_O_BUFS, space="PSUM"))
          ps_f = ps_o if SHARE_PF else stk.enter_context(
              tc.tile_pool(name="ps_f", bufs=PS_F_BUFS, space="PSUM"))
          dram = stk.enter_context(tc.tile_pool(name="dram", bufs=9, space="DRAM"))
          PF_TAG = "ps_o" if SHARE_PF else "ps_f"
          PF_BUFS = PS_O_BUFS if SHARE_PF else PS_F_BUFS
          with (tc.For_i(0, reps, 1) if reps > 1 else contextlib.nullcontext()):
            # ---------------- weights + bias ----------------
            wbs = {}
            for nm, ext in (("wq", wq_e), ("wk", wk_e), ("wv", wv_e)):
                w32 = stage.tile([128, 512], F32, tag="w32", name="w32")
                wb = persist.tile([128, 512], BF16, tag=f"{nm}b", name=f"{nm}b")
                nc.sync.dma_start(
                    w32[:].rearrange("p (c h) -> p c h", h=128),
                    ext[:].rearrange("(c p) h -> p c h", p=128))
                nc.vector.tensor_copy(wb[:], w32[:])
                wbs[nm] = wb
            wqb, wkb, wvb = wbs["wq"], wbs["wk"], wbs["wv"]
            w32 = stage.tile([128, 512], F32, tag="w32", name="w32")
            wob = persist.tile([128, 512], BF16, tag="wob")
            nc.sync.dma_start(w32[:], wo_e[:])
            nc.vector.tensor_copy(wob[:], w32[:])

            bo1 = persist.tile([1, 512], F32, tag="bo1")
            nc.sync.dma_start(bo1[:], bo_e[:])
            bob = persist.tile([128, 512], F32, tag="bob")
            nc.gpsimd.partition_broadcast(bob[:], bo1[:])
            ones64 = persist.tile([1, 64], F32, tag="ones64")
            nc.vector.memset(ones64[:], 1.0)
            if warm_table:
                warm = stage.tile([1, 64], F32, tag="warm", name="warm")
                nc.scalar.activation(warm[:], ones64[:], EXP, scale=0.01)

            # ---------------- load x, cast to bf16 ----------------
            xbf = persist.tile([128, 4 * N], BF16, tag="xbf")  # 4 chunks of 4096
            x_engs = ([nc.sync, nc.gpsimd, nc.scalar, nc.sync] if front_split
                      else [nc.sync, nc.sync, nc.sync, nc.sync])
            for c in range(4):
                x32 = stage.tile([128, N], F32, tag="x32", name="x32")
                x_engs[c].dma_start(x32[:], xT_e[c * 128:(c + 1) * 128, :])
                nc.vector.tensor_copy(xbf[:, c * N:(c + 1) * N], x32[:])

            # ---------------- projections ----------------
            kT = persist.tile([128, N], BF16, tag="kT")
            qT = persist.tile([128, N], BF16, tag="qT")
            vsb = persist.tile([128, NKT * VW], BF16, tag="vsb")

            def proj_chunk(dst, w, f):
                if w512:
                    for half in range(2):
                        ps = ps_big.tile([128, 512], F32, tag="ps_big",
                                         name="ps", bufs=4)
                        for c in range(4):
                            nc.tensor.matmul(
                                ps[:],
                                w[:, c * 128:(c + 1) * 128],
                                xbf[:, c * N + f * 1024 + half * 512:
                                    c * N + f * 1024 + (half + 1) * 512],
                                start=(c == 0), stop=(c == 3),
                            )
                        nc.vector.tensor_copy(
                            dst[:, half * 512:(half + 1) * 512], ps[:])
                    return
                ptag = "ps_b" if act2048 else "ps_big"
                ps = ps_big.tile([128, 1024], F32, tag=ptag, name="ps", bufs=1 if act2048 else PS_BIG_BUFS)
                for half in range(2):
                    for c in range(4):
                        nc.tensor.matmul(
                            ps[:, half * 512:(half + 1) * 512],
                            w[:, c * 128:(c + 1) * 128],
                            xbf[:, c * N + f * 1024 + half * 512:
                                c * N + f * 1024 + (half + 1) * 512],
                            start=(c == 0), stop=(c == 3),
                        )
                nc.vector.tensor_copy(dst[:], ps[:])

            def vproj4(t0, pool, tag, width, vbufs=2):
                # project v tiles t0..t0+3
                ps = pool.tile([128, width], F32, tag=tag, name="vps", bufs=vbufs)
                for i in range(4):
                    t = t0 + i
                    for c in range(4):
                        nc.tensor.matmul(
                            ps[:, i * 128:(i + 1) * 128],
                            xbf[:, c * N + t * 128: c * N + (t + 1) * 128],
                            wvb[:, c * 128:(c + 1) * 128],
                            start=(c == 0), stop=(c == 3),
                        )
                for i in range(4):
                    t = t0 + i
                    nc.vector.tensor_copy(
                        vsb[:, t * VW: t * VW + 64], ps[:, i * 128: i * 128 + 64])
                    nc.vector.tensor_copy(
                        vsb[:, t * VW + 65: t * VW + 129],
                        ps[:, i * 128 + 64: (i + 1) * 128])

            def vproj2(t0):
                ps = ps_big.tile([128, 512], F32, tag="ps_big", name="vps",
                                 bufs=4)
                for i in range(2):
                    t = t0 + i
                    for c in range(4):
                        nc.tensor.matmul(
                            ps[:, i * 128:(i + 1) * 128],
                            xbf[:, c * N + t * 128: c * N + (t + 1) * 128],
                            wvb[:, c * 128:(c + 1) * 128],
                            start=(c == 0), stop=(c == 3),
                        )
                for i in range(2):
                    t = t0 + i
                    nc.vector.tensor_copy(
                        vsb[:, t * VW: t * VW + 64], ps[:, i * 128: i * 128 + 64])
                    nc.vector.tensor_copy(
                        vsb[:, t * VW + 65: t * VW + 129],
                        ps[:, i * 128 + 64: (i + 1) * 128])

            for f in range(4):
                proj_chunk(kT[:, f * 1024:(f + 1) * 1024], wkb, f)
            nc.vector.memset(vsb[:], 1.0)
            if inject:
                proj_chunk(qT[:, 0:1024], wqb, 0)
            else:
                for f in range(4):
                    proj_chunk(qT[:, f * 1024:(f + 1) * 1024], wqb, f)
                if w512:
                    for t0 in range(0, NKT, 2):
                        vproj2(t0)
                else:
                    for t0 in range(0, NKT, 4):
                        vproj4(t0, ps_big, "ps_b" if act2048 else "ps_big", 1024, 1 if act2048 else PS_BIG_BUFS)

            # ---------------- attention + output projection ----------------
            outT = persist.tile([128, N], BF16, tag="outT")
            parts = []
            po_all = {}

            def qk_mm(ps, col, h, kt, q0):
                nc.tensor.matmul(
                    ps[:, col * 512:(col + 1) * 512],
                    kT[h * 64:(h + 1) * 64, kt * 128:(kt + 1) * 128],
                    qT[h * 64:(h + 1) * 64, q0:q0 + 512],
                    start=True, stop=True,
                    tile_position=(64 * h, 0),
                )

            def av_mm(po, es, col, h, kt):
                nc.tensor.matmul(
                    po[:],
                    vsb[:, kt * VW + 65 * h: kt * VW + 65 * h + 65],
                    es[:, col * 512:(col + 1) * 512],
                    start=(kt == 0), stop=(kt == NKT - 1),
                )

            def attn_block(qb):
                q0 = qb * 512
                po_all[qb] = [ps_o.tile([65, 512], F32, tag="ps_o",
                                        name=f"po{qb}_{i}") for i in range(2)]
                po = po_all[qb]
                if act2048:
                    # alternate a 4-bank (2 k-tiles x 2 heads) and a 2-bank
                    # (1 k-tile x 2 heads) score tile; one exp per tile.
                    groups = [(3 * g, 3 * g + 1, 3 * g + 2) for g in range(10)]
                    groups.append((30, 31, None))
                    for ka, kb, kc in groups:
                        psa = ps_big.tile([128, 2048], F32, tag="ps_a",
                                          name="psa", bufs=1)
                        for j, kt in enumerate((ka, kb)):
                            for h in range(2):
                                qk_mm(psa, 2 * j + h, h, kt, q0)
                        esa = esp.tile([128, 2048], BF16, tag="esa", name="esa",
                                       bufs=2)
                        nc.scalar.activation(esa[:], psa[:], EXP, scale=SCALE)
                        for j, kt in enumerate((ka, kb)):
                            for h in range(2):
                                av_mm(po[h], esa, 2 * j + h, h, kt)
                        if kc is None:
                            continue
                        psb = ps_big.tile([128, 1024], F32, tag="ps_b",
                                          name="psb", bufs=1)
                        for h in range(2):
                            qk_mm(psb, h, h, kc, q0)
                        esb = esp.tile([128, 1024], BF16, tag="esb", name="esb",
                                       bufs=2)
                        nc.scalar.activation(esb[:], psb[:], EXP, scale=SCALE)
                        for h in range(2):
                            av_mm(po[h], esb, h, h, kc)
                    return
                if hybrid_exp:
                    # Per 8 k-tiles: the first 4 are staged through SBUF (DVE
                    # copies the f32 scores to a bf16 staging tile; one
                    # 4096-wide exp covers all 4), the last 4 take the direct
                    # PSUM-source 1024-wide exp path. Splits the softmax-exp
                    # overhead between ScalarE and the otherwise-idle VectorE.
                    for b8 in range(0, NKT, 8):
                        stg = esp.tile([128, 4096], BF16, tag="stg",
                                       name="stg", bufs=2)
                        for j, kt in enumerate(range(b8, b8 + 4)):
                            ps = ps_big.tile([128, 1024], F32, tag="ps_big",
                                             name="ps", bufs=PS_BIG_BUFS)
                            for h in range(2):
                                qk_mm(ps, h, h, kt, q0)
                            nc.vector.tensor_copy(
                                stg[:, j * 1024:(j + 1) * 1024], ps[:])
                        esa = esp.tile([128, 4096], BF16, tag="esa",
                                       name="esa", bufs=2)
                        nc.scalar.activation(esa[:], stg[:], EXP, scale=SCALE)
                        for j, kt in enumerate(range(b8, b8 + 4)):
                            for h in range(2):
                                av_mm(po[h], esa, 2 * j + h, h, kt)
                        for kt in range(b8 + 4, b8 + 8):
                            ps = ps_big.tile([128, 1024], F32, tag="ps_big",
                                             name="ps", bufs=PS_BIG_BUFS)
                            for h in range(2):
                                qk_mm(ps, h, h, kt, q0)
                            es = esp.tile([128, 1024], BF16, tag="es",
                                          name="es")
                            nc.scalar.activation(es[:], ps[:], EXP, scale=SCALE)
                            for h in range(2):
                                av_mm(po[h], es, h, h, kt)
                    return
                if w512:
                    for kt in range(NKT):
                        for h in range(2):
                            ps = ps_big.tile([128, 512], F32, tag="ps_big",
                                             name="ps", bufs=4)
                            qk_mm(ps, 0, h, kt, q0)
                            es = esp.tile([128, 512], BF16, tag="es",
                                          name="es", bufs=6)
                            nc.scalar.activation(es[:], ps[:], EXP, scale=SCALE)
                            av_mm(po[h], es, 0, h, kt)
                    return
                for kt in range(NKT):
                    if inject and qb == 0 and kt % 4 == 0:
                        vproj4(kt, ps_f, "ps_f", 512, PS_F_BUFS)
                    if inject and 1 <= qb <= 3 and kt == 4:
                        proj_chunk(qT[:, qb * 1024:(qb + 1) * 1024], wqb, qb)
                    ps = ps_big.tile([128, 1024], F32, tag="ps_big", name="ps",
                                     bufs=PS_BIG_BUFS)
                    for h in range(2):
                        qk_mm(ps, h, h, kt, q0)
                    es = esp.tile([128, 1024], BF16, tag="es", name="es")
                    nc.scalar.activation(es[:], ps[:], EXP, scale=SCALE)
                    for h in range(2):
                        av_mm(po[h], es, h, h, kt)

            def finish_block(qb):
                q0 = qb * 512
                for h in range(2):
                    po = po_all[qb][h]
                    rc = small.tile([1, 512], F32, tag="rc", name="rc")
                    nc.vector.reciprocal(rc[:], po[64:65, :])
                    rb = small.tile([64, 512], F32, tag="rb", name="rb")
                    if bcast == "pe":
                        if deep_bufs:
                            rbp = ps_big.tile([128, 512], F32, tag="ps_big",
                                              name="rbp", bufs=PS_BIG_BUFS)
                        else:
                            rbp = ps_f.tile([128, 512], F32, tag=PF_TAG, name="rbp", bufs=PF_BUFS)
                        nc.tensor.matmul(rbp[0:64, :], ones64[:], rc[:],
                                         start=True, stop=True)
                        nc.vector.tensor_copy(rb[:], rbp[0:64, :])
                    else:
                        nc.gpsimd.partition_broadcast(rb[:], rc[:])
                    nc.vector.tensor_mul(
                        outT[h * 64:(h + 1) * 64, q0:q0 + 512], po[0:64, :], rb[:])

                part = dram.tile([512, DIM], F32, tag="part", name="part")
                for sub in range(4):
                    pf = ps_f.tile([128, 512], F32, tag=PF_TAG, name="pf", bufs=PF_BUFS)
                    nc.tensor.matmul(
                        pf[:], outT[:, q0 + sub * 128: q0 + (sub + 1) * 128],
                        wob[:], start=True, stop=True)
                    fo = fop.tile([128, 512], F32, tag="fo", name="fo")
                    nc.vector.tensor_copy(fo[:], pf[:])
                    nc.sync.dma_start(part[sub * 128:(sub + 1) * 128, :], fo[:])

                if rs_mode == "chunked":
                    rs = dram.tile([128, DIM], F32, tag="rs", name="rs")
                    nc.gpsimd.collective_compute(
                        "ReduceScatter",
                        mybir.AluOpType.add,
                        replica_groups=[[0, 1, 2, 3], [4, 5, 6, 7]],
                        ins=[part.opt()],
                        outs=[rs.opt()],
                    )
                    rsb = fop.tile([128, 512], F32, tag="rsb", name="rsb")
                    nc.sync.dma_start(rsb[:], rs[:])
                    ob = fop.tile([128, 512], F32, tag="ob", name="ob")
                    nc.vector.tensor_add(ob[:], rsb[:], bob[:])
                    nc.sync.dma_start(out_e[qb], ob[:])
                elif rs_mode == "none":
                    rsb = fop.tile([128, 512], F32, tag="rsb", name="rsb")
                    nc.sync.dma_start(rsb[:], part[0:128, :])
                    ob = fop.tile([128, 512], F32, tag="ob", name="ob")
                    nc.vector.tensor_add(ob[:], rsb[:], bob[:])
                    nc.sync.dma_start(out_e[qb], ob[:])
                else:
                    parts.append(part)

            for qb in range(NQB):
                attn_block(qb)
                if pipelined_tail:
                    if qb >= 1:
                        finish_block(qb - 1)
                else:
                    finish_block(qb)
            if pipelined_tail:
                finish_block(NQB - 1)

            if rs_mode == "single":
                big = dram.tile([N, DIM], F32, tag="big")
                for i, p in enumerate(parts):
                    nc.sync.dma_start(big[i * 512:(i + 1) * 512, :], p[:])
                rs = dram.tile([1024, DIM], F32, tag="rsbig")
                nc.gpsimd.collective_compute(
                    "ReduceScatter",
                    mybir.AluOpType.add,
                    replica_groups=[[0, 1, 2, 3], [4, 5, 6, 7]],
                    ins=[big.opt()],
                    outs=[rs.opt()],
                )
                for i in range(8):
                    rsb = fop.tile([128, 512], F32, tag="rsb", name="rsb")
                    nc.sync.dma_start(rsb[:], rs[i * 128:(i + 1) * 128, :])
                    ob = fop.tile([128, 512], F32, tag="ob", name="ob")
                    nc.vector.tensor_add(ob[:], rsb[:], bob[:])
                    nc.sync.dma_start(out_e[i], ob[:])

    nc.compile()
    return nc


# ----------------------------------------------------------------------------
# v2 builder: pipelined prologue + deferred finish tails.
#
# Trace analysis of v1 (453 us total on HW):
#   * ACT (softmax exp) busy 284 us  -> the roofline engine
#   * first exp at 71 us (serial prologue: x DMA -> cast -> all projections)
#   * ~10 us ACT stall at every q-block boundary (finish chain blocked the
#     PE queue: recip -> broadcast mm -> out-proj mm ahead of next block)
#   * ~29 us serial tail after the last exp
#
# v2 changes:
#   * x DMA split into 16 (feature-chunk x seq-quarter) pieces; projections
#     pipelined per quarter; attention starts after quarter 0 (~12 us),
#     remaining quarters' k/v projections injected into q-block 0 between
#     k-tile groups, q-chunks injected into later blocks.
#   * v obtained by projecting in dim-major layout (cheap 512-wide matmuls,
#     same as k) then per-tile 128x128 DMA-xbar transposes into the
#     seq-major [1 | v_h0 | v_h1 | 1] layout the AV matmul needs.
#   * attention accumulators (po) evacuated PSUM->SBUF immediately after the
#     last AV matmul of a block (frees the PSUM bank in ~1.5 us), the whole
#     normalize/out-project chain runs from SBUF afterwards.
#   * finish(qb) is issued AFTER attn_block(qb+1) so its PE instructions
#     (broadcast + out-proj matmuls) land behind the next block's QK/AV
#     stream in the PE queue; its DVE work runs concurrently. ACT never
#     waits at block boundaries.
#   * PSUM banks: scores 2x[128,1024]f32 (4) + po 2x[65,512]f32 (2) +
#     shared proj/broadcast/out-proj ring 2x[128,512]f32 (2) = 8.
# ----------------------------------------------------------------------------
_NONCE_COUNTER = [0]


def _fresh_nonce():
    # The compile/executable caches between jax and the device key on the
    # module I/O signature but NOT on the embedded bass kernel, so two
    # different kernels with identical I/O silently share a stale NEFF.
    # Give every build a unique dummy-input width so any shape-sensitive
    # cache must miss.
    import time
    _NONCE_COUNTER[0] += 1
    return 16 + (int(time.time() * 10) % 49999) * 8 + _NONCE_COUNTER[0]


def _build_v2(rs_mode="chunked", reps=1, inject=True, es_bufs=4, stage_bufs=4,
              pos_bufs=4, exp_w=1024, debug_taps=False):
    from concourse import bass, bacc, tile
    import concourse.mybir as mybir

    F32 = mybir.dt.float32
    BF16 = mybir.dt.bfloat16
    EXP = mybir.ActivationFunctionType.Exp

    nc = bacc.Bacc(None, target_bir_lowering=False, debug=False, num_devices=NCORES)

    nonce_w = _fresh_nonce()
    nonce_e = nc.declare_dram_parameter("nonce", [1, nonce_w], F32,
                                        isOutput=False)
    dbg = {}
    if debug_taps:
        for nm, sh in (("dkT", [128, N]), ("dqT", [128, N]),
                       ("dvsb", [128, 32 * 130]),
                       ("doutT", [128, N]), ("des", [128, 1024])):
            dbg[nm] = nc.declare_dram_parameter(nm, sh, BF16, isOutput=True)
        dbg["dpo"] = nc.declare_dram_parameter("dpo", [2, 65, 512], F32,
                                               isOutput=True)
    xT_e = nc.declare_dram_parameter("xT", [DIM, N], F32, isOutput=False)
    wq_e = nc.declare_dram_parameter("wq", [DIM, 128], F32, isOutput=False)
    wk_e = nc.declare_dram_parameter("wk", [DIM, 128], F32, isOutput=False)
    wv_e = nc.declare_dram_parameter("wv", [DIM, 128], F32, isOutput=False)
    wo_e = nc.declare_dram_parameter("wo", [128, DIM], F32, isOutput=False)
    bo_e = nc.declare_dram_parameter("bo", [1, DIM], F32, isOutput=False)
    out_e = nc.declare_dram_parameter("out", [8, 128, DIM], F32, isOutput=True)

    NKT = N // 128        # 32 k tiles
    NQB = N // 512        # 8 q blocks
    VW = 130              # v tile: [1 | v_h0 (64) | v_h1 (64) | 1]

    import contextlib
    with tile.TileContext(nc) as tc:
        with contextlib.ExitStack() as stk:
          persist = stk.enter_context(tc.tile_pool(name="persist", bufs=1))
          stage = stk.enter_context(tc.tile_pool(name="stage", bufs=stage_bufs))
          esp = stk.enter_context(tc.tile_pool(name="es", bufs=es_bufs))
          small = stk.enter_context(tc.tile_pool(name="small", bufs=4))
          fop = stk.enter_context(tc.tile_pool(name="fo", bufs=3))
          posp = stk.enter_context(tc.tile_pool(name="posp", bufs=pos_bufs))
          ps_big = stk.enter_context(tc.tile_pool(name="ps_big", bufs=2, space="PSUM"))
          ps_o = stk.enter_context(tc.tile_pool(name="ps_o", bufs=2, space="PSUM"))
          ps_f = stk.enter_context(tc.tile_pool(name="ps_f", bufs=2, space="PSUM"))
          dram = stk.enter_context(tc.tile_pool(name="dram", bufs=9, space="DRAM"))
          nonce_sb = persist.tile([1, 16], F32, tag="nonce_sb")
          nc.sync.dma_start(nonce_sb[:], nonce_e[:, 0:16])
          with (tc.For_i(0, reps, 1) if reps > 1 else contextlib.nullcontext()):
            # ---------------- weights + bias ----------------
            wbs = {}
            for nm, ext in (("wq", wq_e), ("wk", wk_e), ("wv", wv_e)):
                w32 = stage.tile([128, 512], F32, tag="w32", name="w32")
                wb = persist.tile([128, 512], BF16, tag=f"{nm}b", name=f"{nm}b")
                nc.sync.dma_start(
                    w32[:].rearrange("p (c h) -> p c h", h=128),
                    ext[:].rearrange("(c p) h -> p c h", p=128))
                nc.vector.tensor_copy(wb[:], w32[:])
                wbs[nm] = wb
            wqb, wkb, wvb = wbs["wq"], wbs["wk"], wbs["wv"]
            w32 = stage.tile([128, 512], F32, tag="w32", name="w32")
            wob = persist.tile([128, 512], BF16, tag="wob")
            nc.sync.dma_start(w32[:], wo_e[:])
            nc.vector.tensor_copy(wob[:], w32[:])

            bo1 = persist.tile([1, 512], F32, tag="bo1")
            nc.sync.dma_start(bo1[:], bo_e[:])
            bob = persist.tile([128, 512], F32, tag="bob")
            nc.gpsimd.partition_broadcast(bob[:], bo1[:])
            ones64 = persist.tile([1, 64], F32, tag="ones64")
            nc.vector.memset(ones64[:], 1.0)

            # persistent buffers
            xbf = persist.tile([128, 4 * N], BF16, tag="xbf")
            kT = persist.tile([128, N], BF16, tag="kT")
            qT = persist.tile([128, N], BF16, tag="qT")
            vsb = persist.tile([128, NKT * VW], BF16, tag="vsb")
            outT = persist.tile([128, N], BF16, tag="outT")
            nc.vector.memset(vsb[:], 1.0)

            # ---------------- x DMAs: 16 pieces, seq-quarter major ----------
            for f in range(4):
                for c in range(4):
                    x32 = stage.tile([128, 1024], F32, tag="x32", name="x32")
                    nc.sync.dma_start(
                        x32[:], xT_e[c * 128:(c + 1) * 128,
                                     f * 1024:(f + 1) * 1024])
                    # stash handle for the cast, issued later in program order
                    wbs[("x32", f, c)] = x32

            def cast_quarter(f):
                for c in range(4):
                    nc.vector.tensor_copy(
                        xbf[:, c * N + f * 1024: c * N + (f + 1) * 1024],
                        wbs[("x32", f, c)][:])

            # ---------------- projection helpers ----------------
            def proj_half(dst, w, f, half):
                # 512 output cols of a [128, N] dim-major projection
                pj = ps_f.tile([128, 512], F32, tag="ps_f", name="pj")
                for c in range(4):
                    nc.tensor.matmul(
                        pj[:],
                        w[:, c * 128:(c + 1) * 128],
                        xbf[:, c * N + f * 1024 + half * 512:
                            c * N + f * 1024 + half * 512 + 512],
                        start=(c == 0), stop=(c == 3),
                    )
                nc.vector.tensor_copy(dst, pj[:])

            def k_quarter(f):
                for half in range(2):
                    proj_half(kT[:, f * 1024 + half * 512:
                                 f * 1024 + half * 512 + 512], wkb, f, half)

            def v_quarter(f):
                # direct seq-major projection: vsb layout [v0 | 1 | v1 | 1],
                # den row 64 for both heads
                for t0 in (8 * f, 8 * f + 4):
                    pj = ps_f.tile([128, 512], F32, tag="ps_f", name="vps")
                    for i in range(4):
                        t = t0 + i
                        for c in range(4):
                            nc.tensor.matmul(
                                pj[:, i * 128:(i + 1) * 128],
                                xbf[:, c * N + t * 128: c * N + (t + 1) * 128],
                                wvb[:, c * 128:(c + 1) * 128],
                                start=(c == 0), stop=(c == 3),
                            )
                    for i in range(4):
                        t = t0 + i
                        nc.vector.tensor_copy(
                            vsb[:, t * VW: t * VW + 64],
                            pj[:, i * 128: i * 128 + 64])
                        nc.vector.tensor_copy(
                            vsb[:, t * VW + 65: t * VW + 129],
                            pj[:, i * 128 + 64: (i + 1) * 128])

            def q_half(f, half):
                proj_half(qT[:, f * 1024 + half * 512:
                             f * 1024 + half * 512 + 512], wqb, f, half)

            # ---------------- attention ----------------
            def qk_mm(ps, col, h, kt, q0):
                nc.tensor.matmul(
                    ps[:, col * 512:(col + 1) * 512],
                    kT[h * 64:(h + 1) * 64, kt * 128:(kt + 1) * 128],
                    qT[h * 64:(h + 1) * 64, q0:q0 + 512],
                    start=True, stop=True,
                    tile_position=(64 * h, 0),
                )

            def av_mm(po, es, col, h, kt):
                # h0 slice: [v0 | 1], h1 slice: [v1 | 1] -> den row 64 for both
                nc.tensor.matmul(
                    po[:],
                    vsb[:, kt * VW + 65 * h: kt * VW + 65 * h + 65],
                    es[:, col * 512:(col + 1) * 512],
                    start=(kt == 0), stop=(kt == NKT - 1),
                )

            def attn_block(qb, injections):
                q0 = qb * 512
                po = [ps_o.tile([65, 512], F32, tag="ps_o",
                                name=f"po{qb}_{h}") for h in range(2)]
                for kt in range(NKT):
                    for fn in injections.get(kt, ()):
                        fn()
                    ps = ps_big.tile([128, 1024], F32, tag="ps_big", name="ps")
                    for h in range(2):
                        qk_mm(ps, h, h, kt, q0)
                    es = esp.tile([128, 1024], BF16, tag="es", name="es")
                    nc.scalar.activation(es[:], ps[:], EXP, scale=SCALE)
                    if debug_taps and qb == 0 and kt == 0:
                        nc.sync.dma_start(dbg["des"][:], es[:])
                    for h in range(2):
                        av_mm(po[h], es, h, h, kt)
                # evacuate accumulators -> SBUF, freeing the po PSUM banks
                poS = [posp.tile([65, 512], F32, tag="poS",
                                 name=f"poS{qb}_{h}") for h in range(2)]
                for h in range(2):
                    nc.vector.tensor_copy(poS[h][:], po[h][:])
                return poS

            def finish_tail(qb, poS):
                q0 = qb * 512
                for h in range(2):
                    rc = small.tile([1, 512], F32, tag="rc", name="rc")
                    nc.vector.reciprocal(rc[:], poS[h][64:65, :])
                    rbp = ps_f.tile([128, 512], F32, tag="ps_f", name="rbp")
                    nc.tensor.matmul(rbp[0:64, :], ones64[:], rc[:],
                                     start=True, stop=True)
                    nc.vector.tensor_mul(
                        outT[h * 64:(h + 1) * 64, q0:q0 + 512],
                        poS[h][0:64, :], rbp[0:64, :])

                part = dram.tile([512, DIM], F32, tag="part", name="part")
                for sub in range(4):
                    pf = ps_f.tile([128, 512], F32, tag="ps_f", name="pf")
                    nc.tensor.matmul(
                        pf[:], outT[:, q0 + sub * 128: q0 + (sub + 1) * 128],
                        wob[:], start=True, stop=True)
                    fo = fop.tile([128, 512], F32, tag="fo", name="fo")
                    nc.vector.tensor_copy(fo[:], pf[:])
                    nc.sync.dma_start(part[sub * 128:(sub + 1) * 128, :], fo[:])

                if rs_mode == "chunked":
                    rs = dram.tile([128, DIM], F32, tag="rs", name="rs")
                    nc.gpsimd.collective_compute(
                        "ReduceScatter",
                        mybir.AluOpType.add,
                        replica_groups=[[0, 1, 2, 3], [4, 5, 6, 7]],
                        ins=[part.opt()],
                        outs=[rs.opt()],
                    )
                    rsb = fop.tile([128, 512], F32, tag="rsb", name="rsb")
                    nc.sync.dma_start(rsb[:], rs[:])
                    ob = fop.tile([128, 512], F32, tag="ob", name="ob")
                    nc.vector.tensor_add(ob[:], rsb[:], bob[:])
                    nc.sync.dma_start(out_e[qb], ob[:])
                else:
                    rsb = fop.tile([128, 512], F32, tag="rsb", name="rsb")
                    nc.sync.dma_start(rsb[:], part[0:128, :])
                    ob = fop.tile([128, 512], F32, tag="ob", name="ob")
                    nc.vector.tensor_add(ob[:], rsb[:], bob[:])
                    nc.sync.dma_start(out_e[qb], ob[:])

            # ---------------- program ----------------
            if inject:
                cast_quarter(0)
                k_quarter(0)
                v_quarter(0)
                q_half(0, 0)
                inj0 = {2: [lambda: q_half(0, 1)]}
                for f in (1, 2, 3):
                    inj0[8 * f] = [
                        (lambda ff: lambda: cast_quarter(ff))(f),
                        (lambda ff: lambda: k_quarter(ff))(f),
                        (lambda ff: lambda: v_quarter(ff))(f),
                    ]
                block_inj = {0: inj0}
                # q chunk f feeds q-blocks 2f and 2f+1; inject during block 2f-1
                for f in (1, 2, 3):
                    block_inj[2 * f - 1] = {
                        8: [(lambda ff: lambda: q_half(ff, 0))(f)],
                        16: [(lambda ff: lambda: q_half(ff, 1))(f)],
                    }
            else:
                for f in range(4):
                    cast_quarter(f)
                    k_quarter(f)
                    v_quarter(f)
                    q_half(f, 0)
                    q_half(f, 1)
                block_inj = {}

            poS_prev = None
            for qb in range(NQB):
                poS = attn_block(qb, block_inj.get(qb, {}))
                if debug_taps and qb == 0:
                    for h in range(2):
                        nc.sync.dma_start(dbg["dpo"][h], poS[h][:])
                if poS_prev is not None:
                    finish_tail(qb - 1, poS_prev)
                poS_prev = poS
            finish_tail(NQB - 1, poS_prev)
            if debug_taps:
                nc.sync.dma_start(dbg["dkT"][:], kT[:])
                nc.sync.dma_start(dbg["dqT"][:], qT[:])
                nc.sync.dma_start(dbg["dvsb"][:], vsb[:])
                nc.sync.dma_start(dbg["doutT"][:], outT[:])

    nc.compile()
    return nc


# Final configuration: v2 (pipelined prologue + deferred finish tails).
FINAL_FLAGS = dict(rs_mode="chunked", inject=True, es_bufs=4)


def build_final(reps=1, **overrides):
    flags = dict(FINAL_FLAGS)
    flags.update(overrides)
    return _build_v2(reps=reps, **flags)


def _get_nc():
    if "nc" not in _CACHE:
        _CACHE["nc"] = build_final()
    return _CACHE["nc"]


# ----------------------------------------------------------------------------
# PJRT runner (mirrors bass2jax.run_bass_via_pjrt multi-core branch, but keeps
# the jitted callable cached so repeated calls / benchmarking don't recompile)
# ----------------------------------------------------------------------------
def _pjrt_exec(nc, in_maps, bench_iters=0, key="runner"):
    import jax
    import numpy as _np
    from jax.sharding import Mesh, PartitionSpec, NamedSharding
    from jax.experimental.shard_map import shard_map
    import concourse.mybir as mybir
    from concourse import bass2jax

    bass2jax.install_neuronx_cc_hook()

    n_cores = NCORES
    if key not in _CACHE:
        pname = nc.partition_id_tensor.name if nc.partition_id_tensor else None
        in_names, out_names, out_avals, zero_outs = [], [], [], []
        for alloc in nc.m.functions[0].allocations:
            if not isinstance(alloc, mybir.MemoryLocationSet):
                continue
            name = alloc.memorylocations[0].name
            if alloc.kind == "ExternalInput":
                if name != pname:
                    in_names.append(name)
            elif alloc.kind == "ExternalOutput":
                sh = tuple(alloc.tensor_shape)
                dt = mybir.dt.np(alloc.dtype)
                out_names.append(name)
                out_avals.append(jax.core.ShapedArray(sh, dt))
                zero_outs.append(_np.zeros(sh, dt))
        n_params = len(in_names)
        n_outs = len(out_avals)
        all_names = in_names + out_names + ([pname] if pname else [])

        def _body(*args):
            operands = list(args)
            if pname is not None:
                operands.append(bass2jax.partition_id_tensor())
            outs = bass2jax._bass_exec_p.bind(
                *operands,
                out_avals=tuple(out_avals),
                in_names=tuple(all_names),
                out_names=tuple(out_names),
                lowering_input_output_aliases=(),
                sim_require_finite=True,
                sim_require_nnan=True,
                nc=nc,
            )
            return tuple(outs)

        # The axon-terminal executable cache can serve stale NEFFs for
        # byte-different HLO modules that share the jit name + signature.
        # Bake a content hash of the kernel into the jit name so every
        # distinct build compiles fresh.
        import hashlib
        _body.__name__ = "body_" + hashlib.sha256(
            nc.to_json_bytes()).hexdigest()[:10]
        _body.__qualname__ = _body.__name__

        donate = tuple(range(n_params, n_params + n_outs))
        devices = jax.devices()[:n_cores]
        mesh = Mesh(_np.asarray(devices), ("core",))
        in_specs = (PartitionSpec("core"),) * (n_params + n_outs)
        out_specs = (PartitionSpec("core"),) * n_outs
        sharded = jax.jit(
            shard_map(_body, mesh=mesh, in_specs=in_specs, out_specs=out_specs,
                      check_rep=False),
            donate_argnums=donate, keep_unused=True)
        _CACHE[key] = (sharded, in_names, out_names, out_avals, zero_outs, mesh)

    sharded, in_names, out_names, out_avals, zero_outs, mesh = _CACHE[key]
    shd = NamedSharding(mesh, PartitionSpec("core"))

    # auto-fill inputs not provided by the caller (e.g. the cache-busting
    # nonce) with zeros of the declared shape
    in_shapes = {}
    for alloc in nc.m.functions[0].allocations:
        import concourse.mybir as mybir
        if isinstance(alloc, mybir.MemoryLocationSet) and alloc.kind == "ExternalInput":
            in_shapes[alloc.memorylocations[0].name] = (
                tuple(alloc.tensor_shape), mybir.dt.np(alloc.dtype))

    def _get(m, nm):
        if nm in m:
            return _np.asarray(m[nm])
        sh, dt = in_shapes[nm]
        return _np.zeros(sh, dt)

    concat_in = [
        jax.device_put(
            _np.concatenate([_get(m, nm) for m in in_maps], axis=0), shd)
        for nm in in_names
    ]
    import jax.numpy as _jnp
    _zfns = [jax.jit(lambda z=z: _jnp.zeros((n_cores * z.shape[0], *z.shape[1:]),
                                            z.dtype), out_shardings=shd)
             for z in zero_outs]
    def zeros_dev():
        return [f() for f in _zfns]

    out_arrs = sharded(*concat_in, *zeros_dev())
    jax.block_until_ready(out_arrs)

    per_iter_ns = None
    if bench_iters > 0:
        import time as _time
        zs = [zeros_dev() for _ in range(bench_iters)]
        # warmup a couple extra dispatches
        for z in zs[:2]:
            o = sharded(*concat_in, *z)
        jax.block_until_ready(o)
        zs = [zeros_dev() for _ in range(bench_iters)]
        jax.block_until_ready(zs)
        t0 = _time.perf_counter()
        for z in zs:
            o = sharded(*concat_in, *z)
        jax.block_until_ready(o)
        t1 = _time.perf_counter()
        per_iter_ns = (t1 - t0) / bench_iters * 1e9

    results = [
        {nm: _np.asarray(out_arrs[i]).reshape(n_cores, *out_avals[i].shape)[c]
         for i, nm in enumerate(out_names)}
        for c in range(n_cores)
    ]
    return results, per_iter_ns


# ----------------------------------------------------------------------------
# Entry point
# ----------------------------------------------------------------------------
def kernel(x, Wq, aq, Wk, ak, Wv, av, Wo, ao, bo):
    global LAST_RESULT

    x = np.asarray(x, dtype=np.float32)
    Qq = cayley_heads_np(np.asarray(Wq), float(aq))
    Qk = cayley_heads_np(np.asarray(Wk), float(ak))
    Qv = cayley_heads_np(np.asarray(Wv), float(av))
    Qo = cayley_heads_np(np.asarray(Wo), float(ao))
    bo = np.asarray(bo, dtype=np.float32)

    nc = _get_nc()

    in_maps = []
    for c in range(NCORES):
        b = c // 4
        hp = c % 4
        sl = slice(hp * 128, (hp + 1) * 128)  # this core's two heads' dims
        in_maps.append({
            "xT": np.ascontiguousarray(x[b].T),                       # (512, 4096)
            "wq": np.ascontiguousarray(Qq[sl].T).astype(np.float32),  # (512, 128)
            "wk": np.ascontiguousarray(Qk[sl].T).astype(np.float32),
            "wv": np.ascontiguousarray(Qv[sl].T).astype(np.float32),
            "wo": np.ascontiguousarray(Qo[:, sl].T).astype(np.float32),  # (128, 512)
            "bo": bo.reshape(1, DIM),
        })

    _CACHE["last_in_maps"] = in_maps
    bench_iters = int(os.environ.get("KERNEL_BENCH", "0"))
    results, per_iter_ns = _pjrt_exec(nc, in_maps, bench_iters=bench_iters)
    LAST_RESULT = {"per_iter_ns": per_iter_ns}

    out = np.empty((B, N, DIM), dtype=np.float32)
    for c in range(NCORES):
        b = c // 4
        r = c % 4
        oc = results[c]["out"]  # (8, 128, 512)
        for qb in range(8):
            out[b, qb * 512 + r * 128: qb * 512 + (r + 1) * 128, :] = oc[qb]
    return out



# revision 22
# speedup vs baseline: 1.1573x; 1.1573x over previous
"""Trainium2 8-core kernel for nn_Attention_55070070670307.

Reference model: per-head Cayley-orthogonalized projections (OrthogonLin)
feeding standard multi-head softmax attention.

  x: (2, 4096, 512) f32, 8 heads x 64 dim, Wq/Wk/Wv/Wo (512,512) + scalars
  aq/ak/av/ao + bias bo.

Strategy:
  * Host: Cayley-orthogonalize the four weight matrices per head (32 tiny
    64x64 solves -- negligible FLOPs, done in float64 numpy).
  * Device sharding: batch-parallel x head-parallel. Core c handles batch
    b = c//4 and heads {2*(c%4), 2*(c%4)+1}. Each core computes q/k/v
    projections for its 2 heads over the whole sequence (4096), full
    softmax attention per head, and the partial output projection
    (contribution of its 128 head-dims to all 512 output features).
  * The 4 cores of each batch group ReduceScatter the partial outputs
    (per 512-row chunk, overlapped with remaining compute), add bias,
    and write disjoint row-slices of the final output.

Device layouts (per core):
  xT   (512, 4096)  x[b] transposed (feature-major)       -> bf16 on chip
  qT/kT (128, 4096)  per-head-dim-major projections, bf16
  v    32 tiles (128n, 130) = [v_h0 | ones | v_h1 | ones] bf16 (ones col
       gives the softmax row-sum for free during the AV matmul)
  scores are computed transposed: sT (128k, 512q) = K_tile @ qT so that
  exp(sT) tiles feed the AV matmul as lhsT with zero transposes.
  Softmax uses the unnormalized trick: out = (exp(s) @ [v|1]); divide by
  the ones-column afterwards. No max-subtraction (scores*0.125 is in
  [-6, 6] comfortably for exp in f32).
"""

import os
import sys

import numpy as np

sys.path.insert(0, "/opt/trn_rl_repo")

HEADS = 8
DIM = 512
DH = 64  # dim per head
N = 4096  # sequence length
B = 2
SCALE = DH ** -0.5
NCORES = 8

F32 = None  # set lazily after mybir import
BF16 = None

_CACHE = {}
LAST_RESULT = None  # BassKernelResults of the most recent run (for test.py)


# ----------------------------------------------------------------------------
# Host-side Cayley orthogonalization (matches reference.cayley_heads, f64)
# ----------------------------------------------------------------------------
def cayley_heads_np(W: np.ndarray, alpha: float) -> np.ndarray:
    W = W.astype(np.float64)
    out, inn = W.shape
    d = inn // HEADS
    Wh = W.reshape(HEADS, d, inn)
    norms = np.sqrt((Wh * Wh).sum(axis=(1, 2), keepdims=True))
    Wn = float(alpha) * Wh / norms
    blocks = []
    I = np.eye(d)
    for j in range(HEADS):
        Wt = Wn[j].T  # (inn, d)
        U, V = Wt[:d], Wt[d:]
        A = U - U.T + V.T @ V
        IpA = I + A
        top = np.linalg.solve(IpA, I - A)
        bot = -2.0 * np.linalg.solve(IpA.T, V.T).T
        blocks.append(np.concatenate([top, bot], axis=0).T)  # (d, inn)
    return np.concatenate(blocks, axis=0)  # (out, inn) f64


# ----------------------------------------------------------------------------
# Device kernel builder (one SPMD graph, 8 cores)
# ----------------------------------------------------------------------------
def _build(rs_mode="chunked", reps=1, front_split=False, warm_table=True,
           pipelined_tail=False, inject=False, bcast="pe", es_bufs=3, fo_bufs=3, act2048=False, hybrid_exp=False, deep_bufs=False, w512=False):
    from concourse import bass, bacc, tile
    import concourse.mybir as mybir

    F32 = mybir.dt.float32
    BF16 = mybir.dt.bfloat16
    EXP = mybir.ActivationFunctionType.Exp

    nc = bacc.Bacc(None, target_bir_lowering=False, debug=False, num_devices=NCORES)

    xT_e = nc.declare_dram_parameter("xT", [DIM, N], F32, isOutput=False)
    wq_e = nc.declare_dram_parameter("wq", [DIM, 128], F32, isOutput=False)
    wk_e = nc.declare_dram_parameter("wk", [DIM, 128], F32, isOutput=False)
    wv_e = nc.declare_dram_parameter("wv", [DIM, 128], F32, isOutput=False)
    wo_e = nc.declare_dram_parameter("wo", [128, DIM], F32, isOutput=False)
    bo_e = nc.declare_dram_parameter("bo", [1, DIM], F32, isOutput=False)
    out_e = nc.declare_dram_parameter("out", [8, 128, DIM], F32, isOutput=True)

    NKT = N // 128        # 32 k tiles
    NQB = N // 512        # 8 q blocks (512 wide)
    VW = 130              # v tile width: 64 + 1 + 64 + 1
    PS_O_BUFS = 3 if pipelined_tail else 2
    PS_F_BUFS = 1 if pipelined_tail else 2
    SHARE_PF = act2048 or deep_bufs
    PS_BIG_BUFS = 3 if deep_bufs else 2

    import contextlib
    with tile.TileContext(nc) as tc:
        with contextlib.ExitStack() as stk:
          persist = stk.enter_context(tc.tile_pool(name="persist", bufs=1))
          stage = stk.enter_context(tc.tile_pool(name="stage", bufs=2))
          esp = stk.enter_context(tc.tile_pool(name="es", bufs=es_bufs))
          small = stk.enter_context(tc.tile_pool(name="small", bufs=3))
          fop = stk.enter_context(tc.tile_pool(name="fo", bufs=fo_bufs))
          ps_big = stk.enter_context(tc.tile_pool(name="ps_big", bufs=PS_BIG_BUFS, space="PSUM"))
          ps_o = stk.enter_context(tc.tile_pool(name="ps_o", bufs=PS_O_BUFS, space="PSUM"))
          ps_f = ps_o if SHARE_PF else stk.enter_context(
              tc.tile_pool(name="ps_f", bufs=PS_F_BUFS, space="PSUM"))
          dram = stk.enter_context(tc.tile_pool(name="dram", bufs=9, space="DRAM"))
          PF_TAG = "ps_o" if SHARE_PF else "ps_f"
          PF_BUFS = PS_O_BUFS if SHARE_PF else PS_F_BUFS
          with (tc.For_i(0, reps, 1) if reps > 1 else contextlib.nullcontext()):
            # ---------------- weights + bias ----------------
            wbs = {}
            for nm, ext in (("wq", wq_e), ("wk", wk_e), ("wv", wv_e)):
                w32 = stage.tile([128, 512], F32, tag="w32", name="w32")
                wb = persist.tile([128, 512], BF16, tag=f"{nm}b", name=f"{nm}b")
                nc.sync.dma_start(
                    w32[:].rearrange("p (c h) -> p c h", h=128),
                    ext[:].rearrange("(c p) h -> p c h", p=128))
                nc.vector.tensor_copy(wb[:], w32[:])
                wbs[nm] = wb
            wqb, wkb, wvb = wbs["wq"], wbs["wk"], wbs["wv"]
            w32 = stage.tile([128, 512], F32, tag="w32", name="w32")
            wob = persist.tile([128, 512], BF16, tag="wob")
            nc.sync.dma_start(w32[:], wo_e[:])
            nc.vector.tensor_copy(wob[:], w32[:])

            bo1 = persist.tile([1, 512], F32, tag="bo1")
            nc.sync.dma_start(bo1[:], bo_e[:])
            bob = persist.tile([128, 512], F32, tag="bob")
            nc.gpsimd.partition_broadcast(bob[:], bo1[:])
            ones64 = persist.tile([1, 64], F32, tag="ones64")
            nc.vector.memset(ones64[:], 1.0)
            if warm_table:
                warm = stage.tile([1, 64], F32, tag="warm", name="warm")
                nc.scalar.activation(warm[:], ones64[:], EXP, scale=0.01)

            # ---------------- load x, cast to bf16 ----------------
            xbf = persist.tile([128, 4 * N], BF16, tag="xbf")  # 4 chunks of 4096
            x_engs = ([nc.sync, nc.gpsimd, nc.scalar, nc.sync] if front_split
                      else [nc.sync, nc.sync, nc.sync, nc.sync])
            for c in range(4):
                x32 = stage.tile([128, N], F32, tag="x32", name="x32")
                x_engs[c].dma_start(x32[:], xT_e[c * 128:(c + 1) * 128, :])
                nc.vector.tensor_copy(xbf[:, c * N:(c + 1) * N], x32[:])

            # ---------------- projections ----------------
            kT = persist.tile([128, N], BF16, tag="kT")
            qT = persist.tile([128, N], BF16, tag="qT")
            vsb = persist.tile([128, NKT * VW], BF16, tag="vsb")

            def proj_chunk(dst, w, f):
                if w512:
                    for half in range(2):
                        ps = ps_big.tile([128, 512], F32, tag="ps_big",
                                         name="ps", bufs=4)
                        for c in range(4):
                            nc.tensor.matmul(
                                ps[:],
                                w[:, c * 128:(c + 1) * 128],
                                xbf[:, c * N + f * 1024 + half * 512:
                                    c * N + f * 1024 + (half + 1) * 512],
                                start=(c == 0), stop=(c == 3),
                            )
                        nc.vector.tensor_copy(
                            dst[:, half * 512:(half + 1) * 512], ps[:])
                    return
                ptag = "ps_b" if act2048 else "ps_big"
                ps = ps_big.tile([128, 1024], F32, tag=ptag, name="ps", bufs=1 if act2048 else PS_BIG_BUFS)
                for half in range(2):
                    for c in range(4):
                        nc.tensor.matmul(
                            ps[:, half * 512:(half + 1) * 512],
                            w[:, c * 128:(c + 1) * 128],
                            xbf[:, c * N + f * 1024 + half * 512:
                                c * N + f * 1024 + (half + 1) * 512],
                            start=(c == 0), stop=(c == 3),
                        )
                nc.vector.tensor_copy(dst[:], ps[:])

            def vproj4(t0, pool, tag, width, vbufs=2):
                # project v tiles t0..t0+3
                ps = pool.tile([128, width], F32, tag=tag, name="vps", bufs=vbufs)
                for i in range(4):
                    t = t0 + i
                    for c in range(4):
                        nc.tensor.matmul(
                            ps[:, i * 128:(i + 1) * 128],
                            xbf[:, c * N + t * 128: c * N + (t + 1) * 128],
                            wvb[:, c * 128:(c + 1) * 128],
                            start=(c == 0), stop=(c == 3),
                        )
                for i in range(4):
                    t = t0 + i
                    nc.vector.tensor_copy(
                        vsb[:, t * VW: t * VW + 64], ps[:, i * 128: i * 128 + 64])
                    nc.vector.tensor_copy(
                        vsb[:, t * VW + 65: t * VW + 129],
                        ps[:, i * 128 + 64: (i + 1) * 128])

            def vproj2(t0):
                ps = ps_big.tile([128, 512], F32, tag="ps_big", name="vps",
                                 bufs=4)
                for i in range(2):
                    t = t0 + i
                    for c in range(4):
                        nc.tensor.matmul(
                            ps[:, i * 128:(i + 1) * 128],
                            xbf[:, c * N + t * 128: c * N + (t + 1) * 128],
                            wvb[:, c * 128:(c + 1) * 128],
                            start=(c == 0), stop=(c == 3),
                        )
                for i in range(2):
                    t = t0 + i
                    nc.vector.tensor_copy(
                        vsb[:, t * VW: t * VW + 64], ps[:, i * 128: i * 128 + 64])
                    nc.vector.tensor_copy(
                        vsb[:, t * VW + 65: t * VW + 129],
                        ps[:, i * 128 + 64: (i + 1) * 128])

            for f in range(4):
                proj_chunk(kT[:, f * 1024:(f + 1) * 1024], wkb, f)
            nc.vector.memset(vsb[:], 1.0)
            if inject:
                proj_chunk(qT[:, 0:1024], wqb, 0)
            else:
                for f in range(4):
                    proj_chunk(qT[:, f * 1024:(f + 1) * 1024], wqb, f)
                if w512:
                    for t0 in range(0, NKT, 2):
                        vproj2(t0)
                else:
                    for t0 in range(0, NKT, 4):
                        vproj4(t0, ps_big, "ps_b" if act2048 else "ps_big", 1024, 1 if act2048 else PS_BIG_BUFS)

            # ---------------- attention + output projection ----------------
            outT = persist.tile([128, N], BF16, tag="outT")
            parts = []
            po_all = {}

            def qk_mm(ps, col, h, kt, q0):
                nc.tensor.matmul(
                    ps[:, col * 512:(col + 1) * 512],
                    kT[h * 64:(h + 1) * 64, kt * 128:(kt + 1) * 128],
                    qT[h * 64:(h + 1) * 64, q0:q0 + 512],
                    start=True, stop=True,
                    tile_position=(64 * h, 0),
                )

            def av_mm(po, es, col, h, kt):
                nc.tensor.matmul(
                    po[:],
                    vsb[:, kt * VW + 65 * h: kt * VW + 65 * h + 65],
                    es[:, col * 512:(col + 1) * 512],
                    start=(kt == 0), stop=(kt == NKT - 1),
                )

            def attn_block(qb):
                q0 = qb * 512
                po_all[qb] = [ps_o.tile([65, 512], F32, tag="ps_o",
                                        name=f"po{qb}_{i}") for i in range(2)]
                po = po_all[qb]
                if act2048:
                    # alternate a 4-bank (2 k-tiles x 2 heads) and a 2-bank
                    # (1 k-tile x 2 heads) score tile; one exp per tile.
                    groups = [(3 * g, 3 * g + 1, 3 * g + 2) for g in range(10)]
                    groups.append((30, 31, None))
                    for ka, kb, kc in groups:
                        psa = ps_big.tile([128, 2048], F32, tag="ps_a",
                                          name="psa", bufs=1)
                        for j, kt in enumerate((ka, kb)):
                            for h in range(2):
                                qk_mm(psa, 2 * j + h, h, kt, q0)
                        esa = esp.tile([128, 2048], BF16, tag="esa", name="esa",
                                       bufs=2)
                        nc.scalar.activation(esa[:], psa[:], EXP, scale=SCALE)
                        for j, kt in enumerate((ka, kb)):
                            for h in range(2):
                                av_mm(po[h], esa, 2 * j + h, h, kt)
                        if kc is None:
                            continue
                        psb = ps_big.tile([128, 1024], F32, tag="ps_b",
                                          name="psb", bufs=1)
                        for h in range(2):
                            qk_mm(psb, h, h, kc, q0)
                        esb = esp.tile([128, 1024], BF16, tag="esb", name="esb",
                                       bufs=2)
                        nc.scalar.activation(esb[:], psb[:], EXP, scale=SCALE)
                        for h in range(2):
                            av_mm(po[h], esb, h, h, kc)
                    return
                if hybrid_exp:
                    # Per 8 k-tiles: the first 4 are staged through SBUF (DVE
                    # copies the f32 scores to a bf16 staging tile; one
                    # 4096-wide exp covers all 4), the last 4 take the direct
                    # PSUM-source 1024-wide exp path. Splits the softmax-exp
                    # overhead between ScalarE and the otherwise-idle VectorE.
                    for b8 in range(0, NKT, 8):
                        stg = esp.tile([128, 4096], BF16, tag="stg",
                                       name="stg", bufs=2)
                        for j, kt in enumerate(range(b8, b8 + 4)):
                            ps = ps_big.tile([128, 1024], F32, tag="ps_big",
                                             name="ps", bufs=PS_BIG_BUFS)
                            for h in range(2):
                                qk_mm(ps, h, h, kt, q0)
                            nc.vector.tensor_copy(
                                stg[:, j * 1024:(j + 1) * 1024], ps[:])
                        esa = esp.tile([128, 4096], BF16, tag="esa",
                                       name="esa", bufs=2)
                        nc.scalar.activation(esa[:], stg[:], EXP, scale=SCALE)
                        for j, kt in enumerate(range(b8, b8 + 4)):
                            for h in range(2):
                                av_mm(po[h], esa, 2 * j + h, h, kt)
                        for kt in range(b8 + 4, b8 + 8):
                            ps = ps_big.tile([128, 1024], F32, tag="ps_big",
                                             name="ps", bufs=PS_BIG_BUFS)
                            for h in range(2):
                                qk_mm(ps, h, h, kt, q0)
                            es = esp.tile([128, 1024], BF16, tag="es",
                                          name="es")
                            nc.scalar.activation(es[:], ps[:], EXP, scale=SCALE)
                            for h in range(2):
                                av_mm(po[h], es, h, h, kt)
                    return
                if w512:
                    for kt in range(NKT):
                        for h in range(2):
                            ps = ps_big.tile([128, 512], F32, tag="ps_big",
                                             name="ps", bufs=4)
                            qk_mm(ps, 0, h, kt, q0)
                            es = esp.tile([128, 512], BF16, tag="es",
                                          name="es", bufs=6)
                            nc.scalar.activation(es[:], ps[:], EXP, scale=SCALE)
                            av_mm(po[h], es, 0, h, kt)
                    return
                for kt in range(NKT):
                    if inject and qb == 0 and kt % 4 == 0:
                        vproj4(kt, ps_f, "ps_f", 512, PS_F_BUFS)
                    if inject and 1 <= qb <= 3 and kt == 4:
                        proj_chunk(qT[:, qb * 1024:(qb + 1) * 1024], wqb, qb)
                    ps = ps_big.tile([128, 1024], F32, tag="ps_big", name="ps",
                                     bufs=PS_BIG_BUFS)
                    for h in range(2):
                        qk_mm(ps, h, h, kt, q0)
                    es = esp.tile([128, 1024], BF16, tag="es", name="es")
                    nc.scalar.activation(es[:], ps[:], EXP, scale=SCALE)
                    for h in range(2):
                        av_mm(po[h], es, h, h, kt)

            def finish_block(qb):
                q0 = qb * 512
                for h in range(2):
                    po = po_all[qb][h]
                    rc = small.tile([1, 512], F32, tag="rc", name="rc")
                    nc.vector.reciprocal(rc[:], po[64:65, :])
                    rb = small.tile([64, 512], F32, tag="rb", name="rb")
                    if bcast == "pe":
                        if deep_bufs:
                            rbp = ps_big.tile([128, 512], F32, tag="ps_big",
                                              name="rbp", bufs=PS_BIG_BUFS)
                        else:
                            rbp = ps_f.tile([128, 512], F32, tag=PF_TAG, name="rbp", bufs=PF_BUFS)
                        nc.tensor.matmul(rbp[0:64, :], ones64[:], rc[:],
                                         start=True, stop=True)
                        nc.vector.tensor_copy(rb[:], rbp[0:64, :])
                    else:
                        nc.gpsimd.partition_broadcast(rb[:], rc[:])
                    nc.vector.tensor_mul(
                        outT[h * 64:(h + 1) * 64, q0:q0 + 512], po[0:64, :], rb[:])

                part = dram.tile([512, DIM], F32, tag="part", name="part")
                for sub in range(4):
                    pf = ps_f.tile([128, 512], F32, tag=PF_TAG, name="pf", bufs=PF_BUFS)
                    nc.tensor.matmul(
                        pf[:], outT[:, q0 + sub * 128: q0 + (sub + 1) * 128],
                        wob[:], start=True, stop=True)
                    fo = fop.tile([128, 512], F32, tag="fo", name="fo")
                    nc.vector.tensor_copy(fo[:], pf[:])
                    nc.sync.dma_start(part[sub * 128:(sub + 1) * 128, :], fo[:])

                if rs_mode == "chunked":
                    rs = dram.tile([128, DIM], F32, tag="rs", name="rs")
                    nc.gpsimd.collective_compute(
                        "ReduceScatter",
                        mybir.AluOpType.add,
                        replica_groups=[[0, 1, 2, 3], [4, 5, 6, 7]],
                        ins=[part.opt()],
                        outs=[rs.opt()],
                    )
                    rsb = fop.tile([128, 512], F32, tag="rsb", name="rsb")
                    nc.sync.dma_start(rsb[:], rs[:])
                    ob = fop.tile([128, 512], F32, tag="ob", name="ob")
                    nc.vector.tensor_add(ob[:], rsb[:], bob[:])
                    nc.sync.dma_start(out_e[qb], ob[:])
                elif rs_mode == "none":
                    rsb = fop.tile([128, 512], F32, tag="rsb", name="rsb")
                    nc.sync.dma_start(rsb[:], part[0:128, :])
                    ob = fop.tile([128, 512], F32, tag="ob", name="ob")
                    nc.vector.tensor_add(ob[:], rsb[:], bob[:])
                    nc.sync.dma_start(out_e[qb], ob[:])
                else:
                    parts.append(part)

            for qb in range(NQB):
                attn_block(qb)
                if pipelined_tail:
                    if qb >= 1:
                        finish_block(qb - 1)
                else:
                    finish_block(qb)
            if pipelined_tail:
                finish_block(NQB - 1)

            if rs_mode == "single":
                big = dram.tile([N, DIM], F32, tag="big")
                for i, p in enumerate(parts):
                    nc.sync.dma_start(big[i * 512:(i + 1) * 512, :], p[:])
                rs = dram.tile([1024, DIM], F32, tag="rsbig")
                nc.gpsimd.collective_compute(
                    "ReduceScatter",
                    mybir.AluOpType.add,
                    replica_groups=[[0, 1, 2, 3], [4, 5, 6, 7]],
                    ins=[big.opt()],
                    outs=[rs.opt()],
                )
                for i in range(8):
                    rsb = fop.tile([128, 512], F32, tag="rsb", name="rsb")
                    nc.sync.dma_start(rsb[:], rs[i * 128:(i + 1) * 128, :])
                    ob = fop.tile([128, 512], F32, tag="ob", name="ob")
                    nc.vector.tensor_add(ob[:], rsb[:], bob[:])
                    nc.sync.dma_start(out_e[i], ob[:])

    nc.compile()
    return nc


# ----------------------------------------------------------------------------
# v2 builder: pipelined prologue + deferred finish tails.
#
# Trace analysis of v1 (453 us total on HW):
#   * ACT (softmax exp) busy 284 us  -> the roofline engine
#   * first exp at 71 us (serial prologue: x DMA -> cast -> all projections)
#   * ~10 us ACT stall at every q-block boundary (finish chain blocked the
#     PE queue: recip -> broadcast mm -> out-proj mm ahead of next block)
#   * ~29 us serial tail after the last exp
#
# v2 changes:
#   * x DMA split into 16 (feature-chunk x seq-quarter) pieces; projections
#     pipelined per quarter; attention starts after quarter 0 (~12 us),
#     remaining quarters' k/v projections injected into q-block 0 between
#     k-tile groups, q-chunks injected into later blocks.
#   * v obtained by projecting in dim-major layout (cheap 512-wide matmuls,
#     same as k) then per-tile 128x128 DMA-xbar transposes into the
#     seq-major [1 | v_h0 | v_h1 | 1] layout the AV matmul needs.
#   * attention accumulators (po) evacuated PSUM->SBUF immediately after the
#     last AV matmul of a block (frees the PSUM bank in ~1.5 us), the whole
#     normalize/out-project chain runs from SBUF afterwards.
#   * finish(qb) is issued AFTER attn_block(qb+1) so its PE instructions
#     (broadcast + out-proj matmuls) land behind the next block's QK/AV
#     stream in the PE queue; its DVE work runs concurrently. ACT never
#     waits at block boundaries.
#   * PSUM banks: scores 2x[128,1024]f32 (4) + po 2x[65,512]f32 (2) +
#     shared proj/broadcast/out-proj ring 2x[128,512]f32 (2) = 8.
# ----------------------------------------------------------------------------
_NONCE_COUNTER = [0]


def _fresh_nonce():
    # The compile/executable caches between jax and the device key on the
    # module I/O signature but NOT on the embedded bass kernel, so two
    # different kernels with identical I/O silently share a stale NEFF.
    # Give every build a unique dummy-input width so any shape-sensitive
    # cache must miss.
    import time
    _NONCE_COUNTER[0] += 1
    return 16 + (int(time.time() * 10) % 49999) * 8 + _NONCE_COUNTER[0]


def _build_v2(rs_mode="chunked", reps=1, inject=True, es_bufs=6, stage_bufs=4,
              pos_bufs=4, exp_w=1024, debug_taps=False):
    from concourse import bass, bacc, tile
    import concourse.mybir as mybir

    F32 = mybir.dt.float32
    BF16 = mybir.dt.bfloat16
    EXP = mybir.ActivationFunctionType.Exp

    nc = bacc.Bacc(None, target_bir_lowering=False, debug=False, num_devices=NCORES)

    nonce_w = _fresh_nonce()
    nonce_e = nc.declare_dram_parameter("nonce", [1, nonce_w], F32,
                                        isOutput=False)
    dbg = {}
    if debug_taps:
        for nm, sh in (("dkT", [128, N]), ("dqT", [128, N]),
                       ("dvsb", [128, 32 * 130]),
                       ("doutT", [128, N]), ("des", [128, 1024])):
            dbg[nm] = nc.declare_dram_parameter(nm, sh, BF16, isOutput=True)
        dbg["dpo"] = nc.declare_dram_parameter("dpo", [2, 65, 512], F32,
                                               isOutput=True)
    xT_e = nc.declare_dram_parameter("xT", [DIM, N], F32, isOutput=False)
    wq_e = nc.declare_dram_parameter("wq", [DIM, 128], F32, isOutput=False)
    wk_e = nc.declare_dram_parameter("wk", [DIM, 128], F32, isOutput=False)
    wv_e = nc.declare_dram_parameter("wv", [DIM, 128], F32, isOutput=False)
    wo_e = nc.declare_dram_parameter("wo", [128, DIM], F32, isOutput=False)
    bo_e = nc.declare_dram_parameter("bo", [1, DIM], F32, isOutput=False)
    out_e = nc.declare_dram_parameter("out", [8, 128, DIM], F32, isOutput=True)

    NKT = N // 128        # 32 k tiles
    NQB = N // 512        # 8 q blocks
    VW = 130              # v tile: [1 | v_h0 (64) | v_h1 (64) | 1]

    import contextlib
    with tile.TileContext(nc) as tc:
        with contextlib.ExitStack() as stk:
          persist = stk.enter_context(tc.tile_pool(name="persist", bufs=1))
          stage = stk.enter_context(tc.tile_pool(name="stage", bufs=stage_bufs))
          esp = stk.enter_context(tc.tile_pool(name="es", bufs=es_bufs))
          small = stk.enter_context(tc.tile_pool(name="small", bufs=4))
          fop = stk.enter_context(tc.tile_pool(name="fo", bufs=3))
          posp = stk.enter_context(tc.tile_pool(name="posp", bufs=pos_bufs))
          ps_big = stk.enter_context(tc.tile_pool(name="ps_big", bufs=2, space="PSUM"))
          ps_o = stk.enter_context(tc.tile_pool(name="ps_o", bufs=2, space="PSUM"))
          ps_f = stk.enter_context(tc.tile_pool(name="ps_f", bufs=2, space="PSUM"))
          dram = stk.enter_context(tc.tile_pool(name="dram", bufs=9, space="DRAM"))
          nonce_sb = persist.tile([1, 16], F32, tag="nonce_sb")
          nc.sync.dma_start(nonce_sb[:], nonce_e[:, 0:16])
          with (tc.For_i(0, reps, 1) if reps > 1 else contextlib.nullcontext()):
            # ---------------- weights + bias ----------------
            wbs = {}
            for nm, ext in (("wq", wq_e), ("wk", wk_e), ("wv", wv_e)):
                w32 = stage.tile([128, 512], F32, tag="w32", name="w32")
                wb = persist.tile([128, 512], BF16, tag=f"{nm}b", name=f"{nm}b")
                nc.sync.dma_start(
                    w32[:].rearrange("p (c h) -> p c h", h=128),
                    ext[:].rearrange("(c p) h -> p c h", p=128))
                nc.vector.tensor_copy(wb[:], w32[:])
                wbs[nm] = wb
            wqb, wkb, wvb = wbs["wq"], wbs["wk"], wbs["wv"]
            w32 = stage.tile([128, 512], F32, tag="w32", name="w32")
            wob = persist.tile([128, 512], BF16, tag="wob")
            nc.sync.dma_start(w32[:], wo_e[:])
            nc.vector.tensor_copy(wob[:], w32[:])

            bo1 = persist.tile([1, 512], F32, tag="bo1")
            nc.sync.dma_start(bo1[:], bo_e[:])
            bob = persist.tile([128, 512], F32, tag="bob")
            nc.gpsimd.partition_broadcast(bob[:], bo1[:])
            # bias/4: folded into each core's pre-ReduceScatter partial
            bob4 = persist.tile([128, 512], F32, tag="bob4")
            nc.vector.tensor_scalar_mul(bob4[:], bob[:], 0.25)
            ones64 = persist.tile([1, 64], BF16, tag="ones64")
            nc.vector.memset(ones64[:], 1.0)

            # persistent buffers
            xbf = persist.tile([128, 4 * N], BF16, tag="xbf")
            kT = persist.tile([128, N], BF16, tag="kT")
            qT = persist.tile([128, N], BF16, tag="qT")
            vsb = persist.tile([128, NKT * VW], BF16, tag="vsb")
            outT = persist.tile([128, N], BF16, tag="outT")
            nc.vector.memset(vsb[:], 1.0)

            # ---------------- x DMAs: 16 pieces, seq-quarter major ----------
            # quarter 0 issues on the (otherwise idle) scalar queue so its 4
            # pieces land in parallel with the sync queue's later quarters
            for f in range(4):
                for c in range(4):
                    x32 = stage.tile([128, 1024], F32, tag="x32", name="x32")
                    eng = nc.scalar if f == 0 else nc.sync
                    eng.dma_start(
                        x32[:], xT_e[c * 128:(c + 1) * 128,
                                     f * 1024:(f + 1) * 1024])
                    # stash handle for the cast, issued later in program order
                    wbs[("x32", f, c)] = x32

            # PE p-state warm-up: ~4 us of dead matmuls so the real
            # projections and block 0 run at full clock
            warm_ps = ps_f.tile([128, 512], F32, tag="ps_f", name="warm_ps")
            for _ in range(12):
                nc.tensor.matmul(warm_ps[:], wkb[:, 0:128], wkb[:, 0:512],
                                 start=True, stop=True)

            def cast_quarter(f):
                for c in range(4):
                    nc.vector.tensor_copy(
                        xbf[:, c * N + f * 1024: c * N + (f + 1) * 1024],
                        wbs[("x32", f, c)][:])

            # ---------------- projection helpers ----------------
            def proj_half(dst, w, f, half):
                # 512 output cols of a [128, N] dim-major projection
                pj = ps_f.tile([128, 512], F32, tag="ps_f", name="pj")
                for c in range(4):
                    nc.tensor.matmul(
                        pj[:],
                        w[:, c * 128:(c + 1) * 128],
                        xbf[:, c * N + f * 1024 + half * 512:
                            c * N + f * 1024 + half * 512 + 512],
                        start=(c == 0), stop=(c == 3),
                    )
                nc.vector.tensor_copy(dst, pj[:])

            def k_quarter(f):
                for half in range(2):
                    proj_half(kT[:, f * 1024 + half * 512:
                                 f * 1024 + half * 512 + 512], wkb, f, half)

            def v_quarter(f):
                # direct seq-major projection: vsb layout [v0 | 1 | v1 | 1],
                # den row 64 for both heads
                for t0 in (8 * f, 8 * f + 4):
                    pj = ps_f.tile([128, 512], F32, tag="ps_f", name="vps")
                    for i in range(4):
                        t = t0 + i
                        for c in range(4):
                            nc.tensor.matmul(
                                pj[:, i * 128:(i + 1) * 128],
                                xbf[:, c * N + t * 128: c * N + (t + 1) * 128],
                                wvb[:, c * 128:(c + 1) * 128],
                                start=(c == 0), stop=(c == 3),
                            )
                    for i in range(4):
                        t = t0 + i
                        nc.vector.tensor_copy(
                            vsb[:, t * VW: t * VW + 64],
                            pj[:, i * 128: i * 128 + 64])
                        nc.vector.tensor_copy(
                            vsb[:, t * VW + 65: t * VW + 129],
                            pj[:, i * 128 + 64: (i + 1) * 128])

            def q_half(f, half):
                proj_half(qT[:, f * 1024 + half * 512:
                             f * 1024 + half * 512 + 512], wqb, f, half)

            # ---------------- attention ----------------
            def qk_mm(ps, col, h, kt, q0):
                nc.tensor.matmul(
                    ps[:, col * 512:(col + 1) * 512],
                    kT[h * 64:(h + 1) * 64, kt * 128:(kt + 1) * 128],
                    qT[h * 64:(h + 1) * 64, q0:q0 + 512],
                    start=True, stop=True,
                    tile_position=(64 * h, 0),
                )

            def av_mm(po, es, col, h, kt):
                # h0 slice: [v0 | 1], h1 slice: [v1 | 1] -> den row 64 for both
                nc.tensor.matmul(
                    po[:],
                    vsb[:, kt * VW + 65 * h: kt * VW + 65 * h + 65],
                    es[:, col * 512:(col + 1) * 512],
                    start=(kt == 0), stop=(kt == NKT - 1),
                )

            def attn_block(qb, injections):
                q0 = qb * 512
                po = [ps_o.tile([65, 512], F32, tag="ps_o",
                                name=f"po{qb}_{h}") for h in range(2)]
                for kt in range(NKT):
                    for fn in injections.get(kt, ()):
                        fn()
                    ps = ps_big.tile([128, 1024], F32, tag="ps_big", name="ps")
                    for h in range(2):
                        qk_mm(ps, h, h, kt, q0)
                    es = esp.tile([128, 1024], BF16, tag="es", name="es")
                    nc.scalar.activation(es[:], ps[:], EXP, scale=SCALE)
                    if debug_taps and qb == 0 and kt == 0:
                        nc.sync.dma_start(dbg["des"][:], es[:])
                    for h in range(2):
                        av_mm(po[h], es, h, h, kt)
                # evacuate accumulators -> SBUF, freeing the po PSUM banks
                poS = [posp.tile([65, 512], F32, tag="poS",
                                 name=f"poS{qb}_{h}") for h in range(2)]
                for h in range(2):
                    nc.vector.tensor_copy(poS[h][:], po[h][:])
                return poS

            def finish_tail(qb, poS):
                q0 = qb * 512
                for h in range(2):
                    rc = small.tile([1, 512], BF16, tag="rc", name="rc")
                    with nc.allow_low_precision(
                            reason="1/den in bf16: 0.4% on the softmax "
                                   "normalizer, well inside the 2e-2 gate"):
                        nc.vector.reciprocal(rc[:], poS[h][64:65, :])
                    rbp = ps_f.tile([128, 512], F32, tag="ps_f", name="rbp")
                    nc.tensor.matmul(rbp[0:64, :], ones64[:], rc[:],
                                     start=True, stop=True)
                    nc.vector.tensor_mul(
                        outT[h * 64:(h + 1) * 64, q0:q0 + 512],
                        poS[h][0:64, :], rbp[0:64, :])

                part = dram.tile([512, DIM], F32, tag="part", name="part")
                for sub in range(4):
                    pf = ps_f.tile([128, 512], F32, tag="ps_f", name="pf")
                    nc.tensor.matmul(
                        pf[:], outT[:, q0 + sub * 128: q0 + (sub + 1) * 128],
                        wob[:], start=True, stop=True)
                    fo = fop.tile([128, 512], F32, tag="fo", name="fo")
                    # bias/4 folded here: the 4-way ReduceScatter sums it
                    # back to the full bias
                    nc.vector.tensor_add(fo[:], pf[:], bob4[:])
                    nc.sync.dma_start(part[sub * 128:(sub + 1) * 128, :], fo[:])

                if rs_mode == "chunked":
                    # bias is already folded in (bob4): after the RS only a
                    # DRAM->DRAM copy remains, kept on the gpsimd queue so
                    # its wait on the collective can't block compute queues
                    rs = dram.tile([128, DIM], F32, tag="rs", name="rs")
                    nc.gpsimd.collective_compute(
                        "ReduceScatter",
                        mybir.AluOpType.add,
                        replica_groups=[[0, 1, 2, 3], [4, 5, 6, 7]],
                        ins=[part.opt()],
                        outs=[rs.opt()],
                    )
                    nc.gpsimd.dma_start(out_e[qb], rs[:])
                else:
                    rsb = fop.tile([128, 512], F32, tag="rsb", name="rsb")
                    nc.sync.dma_start(rsb[:], part[0:128, :])
                    ob = fop.tile([128, 512], F32, tag="ob", name="ob")
                    nc.vector.tensor_add(ob[:], rsb[:], bob[:])
                    nc.sync.dma_start(out_e[qb], ob[:])

            # ---------------- program ----------------
            if inject:
                cast_quarter(0)
                k_quarter(0)
                q_half(0, 0)
                v_quarter(0)
                inj0 = {2: [lambda: q_half(0, 1)]}
                for f in (1, 2, 3):
                    inj0[8 * f] = [
                        (lambda ff: lambda: cast_quarter(ff))(f),
                        (lambda ff: lambda: k_quarter(ff))(f),
                        (lambda ff: lambda: v_quarter(ff))(f),
                    ]
                block_inj = {0: inj0}
                # q chunk f feeds q-blocks 2f and 2f+1; inject during block 2f-1
                for f in (1, 2, 3):
                    block_inj[2 * f - 1] = {
                        8: [(lambda ff: lambda: q_half(ff, 0))(f)],
                        16: [(lambda ff: lambda: q_half(ff, 1))(f)],
                    }
            else:
                for f in range(4):
                    cast_quarter(f)
                    k_quarter(f)
                    q_half(f, 0)
                    q_half(f, 1)
                    v_quarter(f)
                block_inj = {}

            poS_prev = None
            for qb in range(NQB):
                inj = dict(block_inj.get(qb, {}))
                if poS_prev is not None:
                    # issue the previous block's finish mid-stream: its small
                    # PE tail (broadcast + out-proj) lands behind ~6 tiles of
                    # run-ahead, its DVE work runs concurrently, and its
                    # ReduceScatter fires half a block earlier
                    pp = poS_prev
                    qq = qb - 1
                    inj.setdefault(6, []).append(
                        (lambda a, b: lambda: finish_tail(a, b))(qq, pp))
                poS = attn_block(qb, inj)
                if debug_taps and qb == 0:
                    for h in range(2):
                        nc.sync.dma_start(dbg["dpo"][h], poS[h][:])
                poS_prev = poS
            finish_tail(NQB - 1, poS_prev)
            if debug_taps:
                nc.sync.dma_start(dbg["dkT"][:], kT[:])
                nc.sync.dma_start(dbg["dqT"][:], qT[:])
                nc.sync.dma_start(dbg["dvsb"][:], vsb[:])
                nc.sync.dma_start(dbg["doutT"][:], outT[:])

    nc.compile()
    return nc


# Final configuration: v2 (pipelined prologue + deferred finish tails).
FINAL_FLAGS = dict(rs_mode="chunked", inject=True, es_bufs=6)


def build_final(reps=1, **overrides):
    flags = dict(FINAL_FLAGS)
    flags.update(overrides)
    return _build_v2(reps=reps, **flags)


def _get_nc():
    if "nc" not in _CACHE:
        _CACHE["nc"] = build_final()
    return _CACHE["nc"]


# ----------------------------------------------------------------------------
# PJRT runner (mirrors bass2jax.run_bass_via_pjrt multi-core branch, but keeps
# the jitted callable cached so repeated calls / benchmarking don't recompile)
# ----------------------------------------------------------------------------
def _pjrt_exec(nc, in_maps, bench_iters=0, key="runner"):
    import jax
    import numpy as _np
    from jax.sharding import Mesh, PartitionSpec, NamedSharding
    from jax.experimental.shard_map import shard_map
    import concourse.mybir as mybir
    from concourse import bass2jax

    bass2jax.install_neuronx_cc_hook()

    n_cores = NCORES
    if key not in _CACHE:
        pname = nc.partition_id_tensor.name if nc.partition_id_tensor else None
        in_names, out_names, out_avals, zero_outs = [], [], [], []
        for alloc in nc.m.functions[0].allocations:
            if not isinstance(alloc, mybir.MemoryLocationSet):
                continue
            name = alloc.memorylocations[0].name
            if alloc.kind == "ExternalInput":
                if name != pname:
                    in_names.append(name)
            elif alloc.kind == "ExternalOutput":
                sh = tuple(alloc.tensor_shape)
                dt = mybir.dt.np(alloc.dtype)
                out_names.append(name)
                out_avals.append(jax.core.ShapedArray(sh, dt))
                zero_outs.append(_np.zeros(sh, dt))
        n_params = len(in_names)
        n_outs = len(out_avals)
        all_names = in_names + out_names + ([pname] if pname else [])

        def _body(*args):
            operands = list(args)
            if pname is not None:
                operands.append(bass2jax.partition_id_tensor())
            outs = bass2jax._bass_exec_p.bind(
                *operands,
                out_avals=tuple(out_avals),
                in_names=tuple(all_names),
                out_names=tuple(out_names),
                lowering_input_output_aliases=(),
                sim_require_finite=True,
                sim_require_nnan=True,
                nc=nc,
            )
            return tuple(outs)

        # The axon-terminal executable cache can serve stale NEFFs for
        # byte-different HLO modules that share the jit name + signature.
        # Bake a content hash of the kernel into the jit name so every
        # distinct build compiles fresh.
        import hashlib
        _body.__name__ = "body_" + hashlib.sha256(
            nc.to_json_bytes()).hexdigest()[:10]
        _body.__qualname__ = _body.__name__

        donate = tuple(range(n_params, n_params + n_outs))
        devices = jax.devices()[:n_cores]
        mesh = Mesh(_np.asarray(devices), ("core",))
        in_specs = (PartitionSpec("core"),) * (n_params + n_outs)
        out_specs = (PartitionSpec("core"),) * n_outs
        sharded = jax.jit(
            shard_map(_body, mesh=mesh, in_specs=in_specs, out_specs=out_specs,
                      check_rep=False),
            donate_argnums=donate, keep_unused=True)
        _CACHE[key] = (sharded, in_names, out_names, out_avals, zero_outs, mesh)

    sharded, in_names, out_names, out_avals, zero_outs, mesh = _CACHE[key]
    shd = NamedSharding(mesh, PartitionSpec("core"))

    # auto-fill inputs not provided by the caller (e.g. the cache-busting
    # nonce) with zeros of the declared shape
    in_shapes = {}
    for alloc in nc.m.functions[0].allocations:
        import concourse.mybir as mybir
        if isinstance(alloc, mybir.MemoryLocationSet) and alloc.kind == "ExternalInput":
            in_shapes[alloc.memorylocations[0].name] = (
                tuple(alloc.tensor_shape), mybir.dt.np(alloc.dtype))

    def _get(m, nm):
        if nm in m:
            return _np.asarray(m[nm])
        sh, dt = in_shapes[nm]
        return _np.zeros(sh, dt)

    concat_in = [
        jax.device_put(
            _np.concatenate([_get(m, nm) for m in in_maps], axis=0), shd)
        for nm in in_names
    ]
    import jax.numpy as _jnp
    _zfns = [jax.jit(lambda z=z: _jnp.zeros((n_cores * z.shape[0], *z.shape[1:]),
                                            z.dtype), out_shardings=shd)
             for z in zero_outs]
    def zeros_dev():
        return [f() for f in _zfns]

    out_arrs = sharded(*concat_in, *zeros_dev())
    jax.block_until_ready(out_arrs)

    per_iter_ns = None
    if bench_iters > 0:
        import time as _time
        zs = [zeros_dev() for _ in range(bench_iters)]
        # warmup a couple extra dispatches
        for z in zs[:2]:
            o = sharded(*concat_in, *z)
        jax.block_until_ready(o)
        zs = [zeros_dev() for _ in range(bench_iters)]
        jax.block_until_ready(zs)
        t0 = _time.perf_counter()
        for z in zs:
            o = sharded(*concat_in, *z)
        jax.block_until_ready(o)
        t1 = _time.perf_counter()
        per_iter_ns = (t1 - t0) / bench_iters * 1e9

    results = [
        {nm: _np.asarray(out_arrs[i]).reshape(n_cores, *out_avals[i].shape)[c]
         for i, nm in enumerate(out_names)}
        for c in range(n_cores)
    ]
    return results, per_iter_ns


# ----------------------------------------------------------------------------
# Entry point
# ----------------------------------------------------------------------------
def kernel(x, Wq, aq, Wk, ak, Wv, av, Wo, ao, bo):
    global LAST_RESULT

    x = np.asarray(x, dtype=np.float32)
    Qq = cayley_heads_np(np.asarray(Wq), float(aq))
    Qk = cayley_heads_np(np.asarray(Wk), float(ak))
    Qv = cayley_heads_np(np.asarray(Wv), float(av))
    Qo = cayley_heads_np(np.asarray(Wo), float(ao))
    bo = np.asarray(bo, dtype=np.float32)

    nc = _get_nc()

    in_maps = []
    for c in range(NCORES):
        b = c // 4
        hp = c % 4
        sl = slice(hp * 128, (hp + 1) * 128)  # this core's two heads' dims
        in_maps.append({
            "xT": np.ascontiguousarray(x[b].T),                       # (512, 4096)
            "wq": np.ascontiguousarray(Qq[sl].T).astype(np.float32),  # (512, 128)
            "wk": np.ascontiguousarray(Qk[sl].T).astype(np.float32),
            "wv": np.ascontiguousarray(Qv[sl].T).astype(np.float32),
            "wo": np.ascontiguousarray(Qo[:, sl].T).astype(np.float32),  # (128, 512)
            "bo": bo.reshape(1, DIM),
        })

    _CACHE["last_in_maps"] = in_maps
    bench_iters = int(os.environ.get("KERNEL_BENCH", "0"))
    results, per_iter_ns = _pjrt_exec(nc, in_maps, bench_iters=bench_iters)
    LAST_RESULT = {"per_iter_ns": per_iter_ns}

    out = np.empty((B, N, DIM), dtype=np.float32)
    for c in range(NCORES):
        b = c // 4
        r = c % 4
        oc = results[c]["out"]  # (8, 128, 512)
        for qb in range(8):
            out[b, qb * 512 + r * 128: qb * 512 + (r + 1) * 128, :] = oc[qb]
    return out

